# revision 1
# baseline (speedup 1.0000x reference)
"""Trainium2 Bass kernel for nn_BottleneckFusion (STCN memory readout + ResBlock
+ CBAM + PSP + bottleneck), 8-core SPMD.

Sharding: core c -> (batch b = c//2, half h = c%2).
  Phase A (attention): TM split across the pair (4 memory frames each);
    flash-style combine of (unnormalized value, sumexp) via pairwise AllReduce.
  Phase B (convs/CBAM/PSP): row-half split with halo recompute; row windows are
    uniform in local coordinates; the per-core val window is extracted with a
    dynamic (partition-id driven) free-dim slice; tiny pairwise AllGathers for
    the channel-gate stats and PSP pools.

kernel(**inputs) takes the FULL unsharded inputs and returns the FULL output.
"""
import sys

sys.path.insert(0, "/opt/trn_rl_repo")

import numpy as np
import ml_dtypes

import concourse.bass as bass
import concourse.bacc as bacc
import concourse.mybir as mybir
import concourse.tile as tile
from concourse.bass_utils import run_bass_kernel_spmd

BF16 = ml_dtypes.bfloat16
F16 = np.float16
bf = mybir.dt.bfloat16
f16 = mybir.dt.float16
f32 = mybir.dt.float32
AF = mybir.ActivationFunctionType
ALU = mybir.AluOpType
AX = mybir.AxisListType

N_CORES = 8
B, TM, CIN, CK, CV, COUT, H, W = 4, 8, 256, 64, 256, 256, 32, 32
EPS = 1e-5

# local row coordinates: l = image_row - (r0 - 5), l in 0..25
XROWS = 26                 # x window rows (image r0-5 .. r0+20)
CROWS = 22                 # xb/xc/comp local rows (image r0-3 .. r0+18)
PIXPAD = 768               # padded xc free size (22*34=748 -> 768)
PAIRS = [[0, 1], [2, 3], [4, 5], [6, 7]]
UPS = (2, 4, 8)            # upsampled PSP scales
# full pool pyramid offsets [s1, s2, s4, s8] and own-partial offsets
FOFF = {1: 0, 2: 1, 4: 5, 8: 21}
POFF = {1: 0, 2: 1, 4: 3, 8: 11}


def interp_matrix(s_in, s_out=32):
    if s_in == 1:
        return np.ones((s_out, 1), np.float32)
    c = np.arange(s_out) * (s_in - 1) / (s_out - 1)
    lo = np.floor(c).astype(np.int64)
    hi = np.minimum(lo + 1, s_in - 1)
    w = (c - lo).astype(np.float32)
    M = np.zeros((s_out, s_in), np.float32)
    M[np.arange(s_out), lo] += 1.0 - w
    M[np.arange(s_out), hi] += w
    return M


# ---------------------------------------------------------------------------
# Host-side input preparation
# ---------------------------------------------------------------------------

def _pad_hw(a):
    out = np.zeros(a.shape[:-2] + (34, 34), a.dtype)
    out[..., 1:33, 1:33] = a
    return out


def _chw_chunks(a):
    """[256, ...] -> [128, 2, ...] (partition, chunk)."""
    return a.reshape(2, 128, *a.shape[1:]).transpose(
        1, 0, *range(2, a.ndim + 1))


def prep_core_inputs(inputs, core):
    b, h = core // 2, core % 2
    r0 = 16 * h
    g = {}

    f16_q = np.asarray(inputs["f16_q"], np.float32)
    f16_m = np.asarray(inputs["f16_m"], np.float32)
    value_m = np.asarray(inputs["value_m"], np.float32)

    # xm: [128, 2, 4, 34, 34] padded memory frames
    src = f16_m[b, 4 * h: 4 * h + 4]                        # [4, 256, 32, 32]
    src = src.reshape(4, 2, 128, 32, 32).transpose(2, 1, 0, 3, 4)
    g["xm"] = _pad_hw(src).astype(F16)

    # xq: [128, 2, 34, 34] padded query
    q = _chw_chunks(f16_q[b, 0])                            # [128, 2, 32, 32]
    g["xq"] = _pad_hw(q).astype(F16)

    # vT: [128, 32, 256] transposed value
    V = value_m[b][:, 4 * h: 4 * h + 4].reshape(CV, 4096)
    g["vT"] = np.ascontiguousarray(
        V.T.reshape(32, 128, CV).transpose(1, 0, 2)).astype(BF16)  # stays bf16 (matches e)

    # x window q-part: [128, 2, 26, 34]
    qw = np.zeros((128, 2, XROWS, 34), np.float32)
    for l in range(XROWS):
        img = r0 - 5 + l
        if 0 <= img <= 31:
            qw[:, :, l, 1:33] = q[:, :, img, :]
    g["xqb_raw"] = qw.astype(F16)
    g["xqb_relu"] = np.maximum(qw, 0.0).astype(F16)

    pk_w = np.asarray(inputs["pk_w"], np.float32)
    g["pk_wT"] = np.ascontiguousarray(
        pk_w.reshape(CK, 2, 128, 3, 3).transpose(2, 1, 3, 4, 0)).astype(F16)
    pk_b = np.asarray(inputs["pk_b"], np.float32)
    g["pkb2"] = np.concatenate([pk_b, pk_b]).reshape(128, 1).astype(np.float32)

    def conv_lhsT(w, kc):
        co = w.shape[0]
        return np.ascontiguousarray(
            w.reshape(co, kc, 128, 3, 3).transpose(2, 1, 3, 4, 0)).astype(F16)

    g["rb1_wT"] = conv_lhsT(np.asarray(inputs["rb1_w"], np.float32), 4)
    g["rb2_wT"] = conv_lhsT(np.asarray(inputs["rb2_w"], np.float32), 2)
    g["rbd_wT"] = conv_lhsT(np.asarray(inputs["rbd_w"], np.float32), 4)
    g["rb1_b"] = np.asarray(inputs["rb1_b"], np.float32).reshape(2, 128).T.copy()
    g["xb_bias"] = (np.asarray(inputs["rb2_b"], np.float32)
                    + np.asarray(inputs["rbd_b"], np.float32)
                    ).reshape(2, 128).T.copy()

    w1 = np.asarray(inputs["mlp_w1"], np.float32)           # [16, 256]
    g["mlp_w1T"] = np.ascontiguousarray(
        w1.reshape(16, 2, 128).transpose(2, 1, 0)).copy()   # [128, 2, 16]
    g["mlp_b1"] = np.asarray(inputs["mlp_b1"], np.float32).reshape(16, 1).copy()
    g["mlp_w2T"] = np.ascontiguousarray(
        np.asarray(inputs["mlp_w2"], np.float32).T).copy()  # [16, 256]
    g["mlp_b2x2"] = (2.0 * np.asarray(inputs["mlp_b2"], np.float32)
                     ).reshape(2, 128).T.copy()

    spw = np.asarray(inputs["sp_w"], np.float32)[0]       # [2, 7, 7]
    g["spw_r"] = np.ascontiguousarray(
        spw.reshape(14, 7)).astype(np.float16)                # [(ch,dy), dx]
    bn_scale = float(np.asarray(inputs["sp_g"], np.float32)[0]) / float(
        np.sqrt(1.0 + EPS))
    bn_bias = float(np.asarray(inputs["sp_b"], np.float32)[0])
    g["bn_sb"] = np.array([[bn_scale, bn_bias]], np.float32)

    maskT = np.zeros((128, 6, 1), np.float16)
    mask_mean = np.zeros((1, 768), np.float16)
    for pix in range(CROWS * 34):
        img = r0 - 3 + pix // 34
        if 0 <= img <= 31:
            maskT[pix % 128, pix // 128, 0] = 1.0
            mask_mean[0, pix] = 1.0
    g["comp_maskT"] = maskT
    g["mask_mean"] = mask_mean

    pw = np.zeros((128, 2, 4, 64), np.float32)
    for si, s in enumerate((1, 2, 4, 8)):
        wc = np.asarray(inputs[f"psp_w{s}"], np.float32)[:, :, 0, 0]
        scale = 1.0 / ((32 // s) ** 2)
        pw[:, :, si, :] = (wc.T * scale).reshape(2, 128, 64).transpose(1, 0, 2)
    g["psp_wT"] = pw

    # folded upsample operators: Wup[k=(jr*s+jc), si, (r*32+c)] =
    # M[r0+r, jr] * M[c, jc]
    Wup = np.zeros((64, 3, 512), np.float32)
    for si, s in enumerate(UPS):
        M = interp_matrix(s)
        Mrr = M[r0: r0 + 16, :]                 # [16, s]
        for jr in range(s):
            for jc in range(s):
                Wup[jr * s + jc, si, :] = np.outer(Mrr[:, jr],
                                                   M[:, jc]).reshape(512)
    g["Wup"] = Wup

    bott_w = np.asarray(inputs["bott_w"], np.float32)[:, :, 0, 0]
    g["bott_wT"] = np.ascontiguousarray(
        bott_w.reshape(COUT, 4, 128).transpose(2, 1, 0)).astype(F16)
    g["bott_b"] = np.asarray(inputs["bott_b"], np.float32).reshape(2, 128).T.copy()

    rmask = np.zeros((1, XROWS, 34), np.float16)
    for l in range(XROWS):
        if 0 <= r0 - 5 + l <= 31:
            rmask[0, l, :] = 1.0
    g["rmask"] = rmask

    g["ident"] = np.eye(128, dtype=np.float32)
    return g


INPUT_SPECS = [
    ("xm", [128, 2, 4, 34, 34], f16),
    ("xq", [128, 2, 34, 34], f16),
    ("vT", [128, 32, 256], bf),
    ("xqb_raw", [128, 2, XROWS, 34], f16),
    ("xqb_relu", [128, 2, XROWS, 34], f16),
    ("pk_wT", [128, 2, 3, 3, 64], f16),
    ("pkb2", [128, 1], f32),
    ("rb1_wT", [128, 4, 3, 3, 256], f16),
    ("rb2_wT", [128, 2, 3, 3, 256], f16),
    ("rbd_wT", [128, 4, 3, 3, 256], f16),
    ("rb1_b", [128, 2], f32),
    ("xb_bias", [128, 2], f32),
    ("mlp_w1T", [128, 2, 16], f32),
    ("mlp_b1", [16, 1], f32),
    ("mlp_w2T", [16, 256], f32),
    ("mlp_b2x2", [128, 2], f32),
    ("spw_r", [14, 7], f16),
    ("bn_sb", [1, 2], f32),
    ("comp_maskT", [128, 6, 1], f16),
    ("mask_mean", [1, 768], f16),
    ("psp_wT", [128, 2, 4, 64], f32),
    ("Wup", [64, 3, 512], f32),
    ("bott_wT", [128, 4, 256], f16),
    ("bott_b", [128, 2], f32),
    ("ident", [128, 128], f32),
    ("rmask", [1, XROWS, 34], f16),
]


# ---------------------------------------------------------------------------
# Device kernel
# ---------------------------------------------------------------------------

def build(stage="full"):
    nc = bacc.Bacc("TRN2", target_bir_lowering=False, debug=False,
                   num_devices=N_CORES)
    prm = {n: nc.declare_dram_parameter(n, sh, dt, isOutput=False)
           for n, sh, dt in INPUT_SPECS}
    if stage == "A":
        out_prm = nc.declare_dram_parameter("out_a", [257, 1024], f32,
                                            isOutput=True)
    else:
        out_prm = nc.declare_dram_parameter("out", [128, 2, 16, 32], f32,
                                            isOutput=True)
    if stage == "dbg":
        for n, sh, dt in [("dbg_xraw", [128, 4, XROWS, 34], f16),
                          ("dbg_xb", [128, 2, CROWS, 34], f32),
                          ("dbg_gate", [128, 2, 1], f32),
                          ("dbg_sig", [1, 512], f32),
                          ("dbg_fused", [128, 2, 16, 32], f16),
                          ("dbg_pd", [64, 85], f32),
                          ("dbg_pri0", [128, 512], f16),
                          ("dbg_pri1", [128, 512], f16)]:
            prm[n] = nc.declare_dram_parameter(n, sh, dt, isOutput=True)
    with tile.TileContext(nc) as tc:
        _emit(tc, nc, prm, stage, out_prm)
    nc.compile()
    return nc


def _emit(tc, nc, prm, stage, out_prm):
    import contextlib
    es = contextlib.ExitStack()
    with es:
        wpool = es.enter_context(tc.tile_pool(name="wpool", bufs=1))
        apool = es.enter_context(tc.tile_pool(name="apool", bufs=1))
        dram = es.enter_context(tc.tile_pool(name="dram", bufs=1, space="DRAM"))
        aonly_cm = tc.tile_pool(name="aonly", bufs=1)
        aonly = aonly_cm.__enter__()

        def load(name, pool=wpool):
            t = pool.tile(list(prm[name].shape), prm[name].dtype,
                          name=f"{name}_sb")
            nc.sync.dma_start(t[:], prm[name][:])
            return t

        pk_wT = load("pk_wT")
        pkb2 = load("pkb2")
        xm_sb = aonly.tile([128, 2, 4, 34, 34], f16, name="xm_sb")
        for t in range(4):
            nc.sync.dma_start(xm_sb[:, :, t, :, :], prm["xm"][:, :, t, :, :])
        xq_sb = load("xq", aonly)
        vT_sb = load("vT", aonly)

        ones_bf = wpool.tile([128, 1], bf)
        nc.vector.memset(ones_bf[:], 1.0)

        # ================= phase A =================
        mk_sb = aonly.tile([128, 2, 1024], f16)
        qk_sb = aonly.tile([128, 1024], f16)

        with tc.tile_pool(name="psA", bufs=2, space="PSUM") as psA:
            for tp in range(2):
                for n in range(2):
                    pm = psA.tile([128, 512], f32, tag="mkps", name="pm")
                    for par in range(2):
                        t = 2 * tp + par
                        k = 0
                        for j in range(2):
                            for dy in range(3):
                                for dx in range(3):
                                    nc.tensor.matmul(
                                        pm[64 * par: 64 * par + 64, :],
                                        pk_wT[:, j, dy, dx, :],
                                        xm_sb[:, j, t,
                                              n * 16 + dy: n * 16 + dy + 16,
                                              dx: dx + 32],
                                        start=(k == 0), stop=(k == 17),
                                        tile_position=(0, 64 * par),
                                    )
                                    k += 1
                    nc.scalar.activation(
                        mk_sb[:, tp, n * 512: (n + 1) * 512], pm[:, :],
                        AF.Identity, bias=pkb2[:, 0:1])

            for n in range(2):
                pq = psA.tile([64, 512], f32, tag="qkps", name="pq")
                k = 0
                for j in range(2):
                    for dy in range(3):
                        for dx in range(3):
                            nc.tensor.matmul(
                                pq[:, :], pk_wT[:, j, dy, dx, :],
                                xq_sb[:, j, n * 16 + dy: n * 16 + dy + 16,
                                      dx: dx + 32],
                                start=(k == 0), stop=(k == 17))
                            k += 1
                nc.scalar.activation(
                    qk_sb[0:64, n * 512: (n + 1) * 512], pq[:, :],
                    AF.Identity, bias=pkb2[0:64, 0:1])
            # replicate qk to partitions 64..127 so odd-frame mk slices
            # (base partition 64) can stream against it
            nc.sync.dma_start(qk_sb[64:128, :], qk_sb[0:64, :])

        arv = dram.tile([257, 1024], bf)
        arvg = dram.tile([2, 257, 1024], bf)

        with (
            tc.tile_pool(name="psAff", bufs=2, space="PSUM") as psAff,
            tc.tile_pool(name="psV", bufs=1, space="PSUM") as psV,
        ):
            vps = [psV.tile([128, 1024], f32, name=f"vps{j}") for j in range(2)]
            s_acc = aonly.tile([128, 1024], f32, name="s_acc")

            order = [16 * h + o + 8 * par for h in range(2) for o in range(8)
                     for par in range(2)]
            for idx, i in enumerate(order):
                t = i >> 3
                pb = i & 7
                tp, par = t >> 1, t & 1
                lhs_aff = mk_sb[64 * par: 64 * par + 64, tp,
                                pb * 128: pb * 128 + 128]
                e_t = aonly.tile([128, 1024], bf, tag="e", name="e_t", bufs=3)
                for qn in range(2):
                    pa = psAff.tile([128, 512], f32, tag="affp", name="pa")
                    nc.tensor.matmul(
                        pa[:, :], lhs_aff,
                        qk_sb[64 * par: 64 * par + 64,
                              qn * 512: (qn + 1) * 512],
                        start=True, stop=True)
                    nc.scalar.activation(
                        e_t[:, qn * 512: (qn + 1) * 512], pa[:, :],
                        AF.Exp, scale=0.125)
                for j in range(2):
                    for qn in range(2):
                        nc.tensor.matmul(
                            vps[j][:, qn * 512: (qn + 1) * 512],
                            vT_sb[:, i, j * 128: (j + 1) * 128],
                            e_t[:, qn * 512: (qn + 1) * 512],
                            start=(idx == 0), stop=(idx == 31))
                if idx == 0:
                    nc.vector.tensor_copy(s_acc[:, :], e_t[:, :])
                else:
                    nc.vector.tensor_add(s_acc[:, :], s_acc[:, :], e_t[:, :])

            v_sb = aonly.tile([128, 2, 1024], bf, name="v_sb")
            s_sb = aonly.tile([1, 1024], bf, name="s_sb")
            for j in range(2):
                nc.vector.tensor_copy(v_sb[:, j, :], vps[j][:, :])
                nc.sync.dma_start(arv[128 * j: 128 * j + 128, :], v_sb[:, j, :])
            # fold the 128-partition sumexp accumulator with a ones matmul
            ones_f32 = aonly.tile([128, 1], f32, name="ones_f32")
            nc.vector.memset(ones_f32[:], 1.0)
            sfold = psV.tile([1, 1024], f32, tag="sfold", name="sfold")
            for qn in range(2):
                nc.tensor.matmul(sfold[0:1, qn * 512: (qn + 1) * 512],
                                 ones_f32[:, 0:1],
                                 s_acc[:, qn * 512: (qn + 1) * 512],
                                 start=True, stop=True)
            nc.vector.tensor_copy(s_sb[:, :], sfold[:, :])
            nc.sync.dma_start(arv[256:257, :], s_sb[:, :])

        nc.gpsimd.collective_compute(
            "AllGather", ALU.bypass, replica_groups=PAIRS,
            ins=[arv[:].opt()], outs=[arvg[:].opt()])

        aonly_cm.__exit__(None, None, None)

        if stage == "A":
            with tc.tile_pool(name="cmb", bufs=1) as cmb:
                cs0 = cmb.tile([1, 1024], bf, name="cs0")
                cs1 = cmb.tile([1, 1024], bf, name="cs1")
                cso = cmb.tile([1, 1024], f32, name="cso")
                nc.sync.dma_start(cs0[:], arvg[0, 256:257, :])
                nc.sync.dma_start(cs1[:], arvg[1, 256:257, :])
                nc.vector.tensor_add(cso[:, :], cs0[:, :], cs1[:, :])
                nc.sync.dma_start(out_prm[256:257, :], cso[:, :])
                for j in range(2):
                    ca = cmb.tile([128, 1024], bf, tag="ca", name="ca")
                    cb = cmb.tile([128, 1024], bf, tag="cb", name="cb")
                    co = cmb.tile([128, 1024], f32, tag="co", name="co")
                    nc.sync.dma_start(ca[:, :], arvg[0, 128 * j: 128 * j + 128, :])
                    nc.sync.dma_start(cb[:, :], arvg[1, 128 * j: 128 * j + 128, :])
                    nc.vector.tensor_add(co[:, :], ca[:, :], cb[:, :])
                    nc.sync.dma_start(out_prm[128 * j: 128 * j + 128, :],
                                      co[:, :])
            return

        # ================= phase B =================
        wk = es.enter_context(tc.tile_pool(name="wk", bufs=1))
        rb1_wT = load("rb1_wT")
        rb2_wT = load("rb2_wT")
        rbd_wT = load("rbd_wT")
        rb1_b = load("rb1_b")
        xb_bias = load("xb_bias")
        mlp_w1T = load("mlp_w1T")
        mlp_b1 = load("mlp_b1")
        mlp_w2T = load("mlp_w2T")
        mlp_b2x2 = load("mlp_b2x2")
        spw_r = load("spw_r")
        bn_sb = load("bn_sb")
        comp_maskT = load("comp_maskT")
        mask_mean = load("mask_mean")
        psp_wT = load("psp_wT")
        Wup = load("Wup")
        bott_wT = load("bott_wT")
        bott_b = load("bott_b")
        ident = load("ident")

        r0v = (nc.vector.partition_id() % 2) * 16

        # ---- val: combine AG slots, normalize, window into x ----
        val_pad = apool.tile([128, 2, 42, 32], f32)
        nc.vector.memset(val_pad[:, :, 0:5, :], 0.0)
        nc.vector.memset(val_pad[:, :, 37:42, :], 0.0)
        vs0 = wk.tile([128, 2, 1024], bf, name="vs0")
        vs1 = wk.tile([128, 2, 1024], bf, name="vs1")
        for j in range(2):
            nc.sync.dma_start(vs0[:, j, :], arvg[0, 128 * j: 128 * j + 128, :])
            nc.sync.dma_start(vs1[:, j, :], arvg[1, 128 * j: 128 * j + 128, :])
        for j in range(2):
            nc.vector.tensor_add(
                val_pad[:, j, 5:37, :].rearrange("p r c -> p (r c)"),
                vs0[:, j, :], vs1[:, j, :])
        s0b = wk.tile([128, 1024], bf, name="s0b")
        s1b = wk.tile([128, 1024], bf, name="s1b")
        nc.sync.dma_start(s0b[:], arvg[0, 256:257, :].partition_broadcast(128))
        nc.sync.dma_start(s1b[:], arvg[1, 256:257, :].partition_broadcast(128))
        s_tot = wk.tile([128, 1024], f32, name="s_tot")
        nc.vector.tensor_add(s_tot[:, :], s0b[:, :], s1b[:, :])
        invb = apool.tile([128, 32, 32], f32)
        nc.vector.reciprocal(invb.rearrange("p r c -> p (r c)"), s_tot[:, :])
        for j in range(2):
            nc.vector.tensor_mul(val_pad[:, j, 5:37, :],
                                 val_pad[:, j, 5:37, :], invb[:, :, :])

        x_raw = apool.tile([128, 4, XROWS, 34], f16)
        x_relu = apool.tile([128, 4, XROWS, 34], f16)
        for tt in (x_raw, x_relu):
            nc.vector.memset(tt[:, 2:4, :, 0:1], 0.0)
            nc.vector.memset(tt[:, 2:4, :, 33:34], 0.0)
        nc.sync.dma_start(x_raw[:, 0:2, :, :], prm["xqb_raw"][:])
        nc.sync.dma_start(x_relu[:, 0:2, :, :], prm["xqb_relu"][:])
        for j in range(2):
            nc.vector.tensor_copy(x_raw[:, 2 + j, :, 1:33],
                                  val_pad[:, j, bass.ds(r0v, XROWS), :])
            nc.vector.tensor_relu(x_relu[:, 2 + j, :, 1:33],
                                  val_pad[:, j, bass.ds(r0v, XROWS), :])

        # ---- ResBlock ----
        r1_relu = apool.tile([128, 2, XROWS, 34], f16)
        nc.vector.memset(r1_relu[:, :, 0:1, :], 0.0)
        nc.vector.memset(r1_relu[:, :, 25:26, :], 0.0)
        nc.vector.memset(r1_relu[:, :, :, 0:1], 0.0)
        nc.vector.memset(r1_relu[:, :, :, 33:34], 0.0)
        rmaskb = apool.tile([128, XROWS, 34], f16)
        nc.sync.dma_start(rmaskb[:], prm["rmask"][:].partition_broadcast(128))
        xb = apool.tile([128, 2, PIXPAD], f32)
        xbv = [xb[:, j, 0: CROWS * 34].rearrange("p (r c) -> p r c", c=34)
               for j in range(2)]
        for j in range(2):
            nc.vector.memset(xbv[j][:, :, 0:1], 0.0)
            nc.vector.memset(xbv[j][:, :, 33:34], 0.0)
        nc.vector.memset(xb[:, :, CROWS * 34:], 0.0)
        with tc.tile_pool(name="psB", bufs=2, space="PSUM") as psB:
            for m in range(2):
                for (l0, nr) in ((1, 16), (17, 8)):
                    pr = psB.tile([128, 512], f32, tag="r1ps", name="pr")
                    k = 0
                    for j in range(4):
                        for dy in range(3):
                            for dx in range(3):
                                nc.tensor.matmul(
                                    pr[:, : nr * 32],
                                    rb1_wT[:, j, dy, dx,
                                           m * 128: m * 128 + 128],
                                    x_relu[:, j, l0 + dy - 1: l0 + dy - 1 + nr,
                                           dx: dx + 32],
                                    start=(k == 0), stop=(k == 35))
                                k += 1
                    nc.scalar.activation(
                        r1_relu[:, m, l0: l0 + nr, 1:33], pr[:, : nr * 32],
                        AF.Relu, bias=rb1_b[:, m: m + 1])
                    nc.vector.tensor_mul(r1_relu[:, m, l0: l0 + nr, 1:33],
                                         r1_relu[:, m, l0: l0 + nr, 1:33],
                                         rmaskb[:, l0: l0 + nr, 1:33])

            for m in range(2):
                for (l0, nr) in ((2, 16), (18, 6)):
                    px = psB.tile([128, 512], f32, tag="xbps", name="px")
                    k = 0
                    for j in range(4):
                        for dy in range(3):
                            for dx in range(3):
                                nc.tensor.matmul(
                                    px[:, : nr * 32],
                                    rbd_wT[:, j, dy, dx,
                                           m * 128: m * 128 + 128],
                                    x_raw[:, j, l0 + dy - 1: l0 + dy - 1 + nr,
                                          dx: dx + 32],
                                    start=(k == 0), stop=False)
                                k += 1
                    for j in range(2):
                        for dy in range(3):
                            for dx in range(3):
                                nc.tensor.matmul(
                                    px[:, : nr * 32],
                                    rb2_wT[:, j, dy, dx,
                                           m * 128: m * 128 + 128],
                                    r1_relu[:, j,
                                            l0 + dy - 1: l0 + dy - 1 + nr,
                                            dx: dx + 32],
                                    start=False, stop=(k == 53))
                                k += 1
                    nc.scalar.activation(
                        xbv[m][:, l0 - 2: l0 - 2 + nr, 1:33], px[:, : nr * 32],
                        AF.Identity, bias=xb_bias[:, m: m + 1])

        if stage == "dbg":
            nc.sync.dma_start(prm["dbg_xraw"][:], x_raw[:])
            for j in range(2):
                nc.sync.dma_start(prm["dbg_xb"][:, j], xbv[j])

        # ---- CBAM channel gate ----
        stats = wk.tile([128, 2, 2], f32, name="stats")
        for j in range(2):
            nc.vector.tensor_reduce(stats[:, j, 0:1], xbv[j][:, 3:19, 1:33],
                                    AX.XY, ALU.add)
            nc.vector.tensor_reduce(stats[:, j, 1:2], xbv[j][:, 3:19, 1:33],
                                    AX.XY, ALU.max)
        # transpose xb chunks now (independent of the gate) so the PE work
        # hides under the stats AllGather
        xbT = wk.tile([128, 6, 256], f32, name="xbT")
        with tc.tile_pool(name="psT", bufs=2, space="PSUM") as psT:
            for ch in range(6):
                pt = psT.tile([128, 256], f32, tag="pt", name="pt")
                for j in range(2):
                    nc.tensor.transpose(
                        pt[:, j * 128: j * 128 + 128],
                        xb[:, j, ch * 128: ch * 128 + 128], ident[:, :])
                nc.vector.tensor_copy(xbT[:, ch, :], pt[:, :])

        stats_d = dram.tile([256, 2], f32)
        stats_o = dram.tile([2, 256, 2], f32)
        nc.sync.dma_start(stats_d.rearrange("(j p) k -> p j k", j=2),
                          stats[:, :, :])
        nc.gpsimd.collective_compute(
            "AllGather", ALU.bypass, replica_groups=PAIRS,
            ins=[stats_d[:].opt()], outs=[stats_o[:].opt()])
        sl0 = wk.tile([128, 2, 2], f32, name="sl0")   # [p, j, (sum,max)]
        sl1 = wk.tile([128, 2, 2], f32, name="sl1")
        nc.sync.dma_start(sl0[:, :, :],
                          stats_o[0].rearrange("(j p) k -> p j k", j=2))
        nc.sync.dma_start(sl1[:, :, :],
                          stats_o[1].rearrange("(j p) k -> p j k", j=2))
        gate_in = wk.tile([128, 2, 2], f32, name="gate_in")
        tsum = wk.tile([128, 2, 1], f32, name="tsum")
        nc.vector.tensor_add(tsum[:, :, :], sl0[:, :, 0:1], sl1[:, :, 0:1])
        nc.scalar.mul(gate_in[:, :, 0:1], tsum[:, :, :], 1.0 / 1024.0)
        nc.vector.tensor_max(gate_in[:, :, 1:2], sl0[:, :, 1:2],
                             sl1[:, :, 1:2])

        gate = wk.tile([128, 2, 1], f32, name="gate")
        with tc.tile_pool(name="psG", bufs=1, space="PSUM") as psG:
            ph1 = psG.tile([16, 2], f32, name="ph1")
            for j in range(2):
                nc.tensor.matmul(ph1[:, :], mlp_w1T[:, j, :], gate_in[:, j, :],
                                 start=(j == 0), stop=(j == 1))
            h1 = wk.tile([16, 2], f32, name="h1")
            nc.scalar.activation(h1[:, :], ph1[:, :], AF.Relu,
                                 bias=mlp_b1[:, 0:1])
            for j in range(2):
                ph2 = psG.tile([128, 2], f32, tag="ph2", name="ph2")
                nc.tensor.matmul(ph2[:, :], mlp_w2T[:, j * 128: j * 128 + 128],
                                 h1[:, :], start=True, stop=True)
                h2 = wk.tile([128, 2], f32, tag="h2", name="h2")
                nc.vector.tensor_copy(h2[:, :], ph2[:, :])
                t2 = wk.tile([128, 1], f32, tag="t2", name="t2")
                nc.vector.tensor_add(t2[:, :], h2[:, 0:1], h2[:, 1:2])
                nc.scalar.activation(gate[:, j, :], t2[:, :], AF.Sigmoid,
                                     bias=mlp_b2x2[:, j: j + 1])

        if stage == "dbg":
            nc.sync.dma_start(prm["dbg_gate"][:], gate[:])

        # gate broadcast along pixels (via DRAM re-read) + comp stats
        gate_d = dram.tile([256], f32)
        nc.sync.dma_start(
            bass.AP(gate_d.tensor, 0, [[1, 128], [128, 2]]), gate[:, :, 0])
        gb = wk.tile([128, 256], f32, name="gb")
        nc.sync.dma_start(
            gb[:, :], bass.AP(gate_d.tensor, 0, [[0, 128], [1, 256]]))
        gate_sc = wk.tile([128, 2, 1], f32, name="gate_sc")
        nc.scalar.mul(gate_sc[:, :, :], gate[:, :, :], 1.0 / 256.0)

        # channel max of xb*gate from the transposed copies
        compT = wk.tile([128, 6, 1], f16, name="compT")
        scr = wk.tile([128, 256], f32, name="scr")
        for ch in range(6):
            nc.vector.tensor_mul(scr[:, :], xbT[:, ch, :], gb[:, :])
            nc.vector.tensor_reduce(compT[:, ch, :], scr[:, :], AX.X, ALU.max)
        nc.vector.tensor_mul(compT[:, :, :], compT[:, :, :],
                             comp_maskT[:, :, :])

        # channel mean of xb*gate via gate-weighted ones-matmul
        mean_sb = wk.tile([1, 748], f16, name="mean_sb")
        with tc.tile_pool(name="psM", bufs=1, space="PSUM") as psM:
            pm1 = psM.tile([1, 748], f32, name="pm1")
            for j in range(2):
                for (o0, nn) in ((0, 512), (512, 236)):
                    nc.tensor.matmul(pm1[0:1, o0: o0 + nn],
                                     gate_sc[:, j, :],
                                     xb[:, j, o0: o0 + nn],
                                     start=(j == 0), stop=(j == 1))
            nc.scalar.copy(mean_sb[:, :], pm1[:, :])
        nc.vector.tensor_mul(mean_sb[:, :], mean_sb[:, :],
                             mask_mean[:, 0:748])

        comp_flat = dram.tile([2, 768], f16)
        nc.sync.dma_start(
            comp_flat[0].rearrange("(ch p) -> p ch", ch=6), compT[:, :, 0])
        nc.sync.dma_start(comp_flat[1, 0:748], mean_sb[:, :])
        comp_d = dram.tile([2, CROWS, 38], f16)
        zz = wk.tile([2, CROWS * 38], f16, name="zz")
        nc.vector.memset(zz[:], 0.0)
        nc.sync.dma_start(comp_d.rearrange("s r c -> s (r c)"), zz[:, :])
        nc.sync.dma_start(
            comp_d[:, :, 2:36],
            comp_flat[:, 0:748].rearrange("s (r c) -> s r c", c=34))
        il = wk.tile([14, 16, 38], f16, name="il")
        nc.sync.dma_start(
            il[:, :, :],
            bass.AP(comp_d.tensor, 0,
                    [[836, 2], [38, 7], [38, 16], [1, 38]]))
        sig = wk.tile([1, 512], f32, name="sig")
        with tc.tile_pool(name="psS", bufs=1, space="PSUM") as psS:
            pss = psS.tile([1, 512], f32, name="pss")
            for dx in range(7):
                nc.tensor.matmul(pss[:, :], spw_r[:, dx: dx + 1],
                                 il[:, :, dx: dx + 32],
                                 start=(dx == 0), stop=(dx == 6))
            nc.scalar.activation(sig[:, :], pss[:, :], AF.Sigmoid,
                                 scale=bn_sb[0:1, 0:1], bias=bn_sb[0:1, 1:2])
        sig_d = dram.tile([1, 512], f32)
        nc.sync.dma_start(sig_d[:], sig[:, :])
        sigb = wk.tile([128, 16, 32], f32, name="sigb")
        nc.sync.dma_start(sigb[:], sig_d.partition_broadcast(128))

        if stage == "dbg":
            nc.sync.dma_start(prm["dbg_sig"][:], sig[:])

        # fused = xb_own + (xb_own * gate) * sigb
        fused = apool.tile([128, 2, 16, 32], f16)
        for j in range(2):
            xc_own = wk.tile([128, 16, 32], f32, tag="xc_own", name="xc_own")
            nc.scalar.mul(xc_own[:, :, :], xbv[j][:, 3:19, 1:33],
                          gate[:, j, 0:1])
            tm = wk.tile([128, 16, 32], f32, tag="tm", name="tm")
            nc.vector.tensor_mul(tm[:, :, :], xc_own[:, :, :], sigb[:, :, :])
            nc.vector.tensor_add(fused[:, j, :, :], xbv[j][:, 3:19, 1:33],
                                 tm[:, :, :])

        if stage == "dbg":
            nc.sync.dma_start(prm["dbg_fused"][:], fused[:])

        # ---- PSP pools (raw block sums over own rows) ----
        pools = wk.tile([128, 2, 43], f32, name="pools")
        for j in range(2):
            f8 = fused[:, j].rearrange("p (rb ri) (cb ci) -> p rb cb ri ci",
                                       ri=4, ci=4)
            p8v = pools[:, j, 11:43].rearrange("p (rb cb) -> p rb cb", cb=8)
            nc.vector.tensor_reduce(p8v, f8, AX.XY, ALU.add)
            p8i = pools[:, j, 11:43].rearrange(
                "p (rb ri cb ci) -> p rb cb ri ci", rb=2, ri=2, cb=4, ci=2)
            p4v = pools[:, j, 3:11].rearrange("p (rb cb) -> p rb cb", cb=4)
            nc.vector.tensor_reduce(p4v, p8i, AX.XY, ALU.add)
        p4i = pools[:, :, 3:11].rearrange(
            "p j (rb cb ci) -> p j cb rb ci", rb=2, cb=2, ci=2)
        nc.vector.tensor_reduce(
            pools[:, :, 1:3].rearrange("p j (a k) -> p j a k", a=2, k=1),
            p4i, AX.XY, ALU.add)
        nc.vector.tensor_reduce(pools[:, :, 0:1], pools[:, :, 1:3], AX.X,
                                ALU.add)

        pools_d = dram.tile([2, 128, 43], f32)
        pools_o = dram.tile([2, 2, 128, 43], f32)
        nc.sync.dma_start(pools_d.rearrange("j p k -> p j k"), pools[:, :, :])
        nc.gpsimd.collective_compute(
            "AllGather", ALU.bypass, replica_groups=PAIRS,
            ins=[pools_d[:].opt()], outs=[pools_o[:].opt()])
        slp = [wk.tile([128, 2, 43], f32, tag=f"slp{s}", name=f"slp{s}")
               for s in range(2)]
        for s in range(2):
            nc.sync.dma_start(slp[s][:, :, :],
                              pools_o[s].rearrange("j p k -> p j k"))
        # 1x1 convs on pools: pd [64, 85], column layout [s8|s1|s4|s2]
        # so each scale's transposed block lands at a legal base partition
        PDOFF = {8: 0, 1: 64, 4: 65, 2: 81}
        SI = {1: 0, 2: 1, 4: 2, 8: 3}
        pd = wk.tile([64, 85], f32, name="pd")
        with tc.tile_pool(name="psP", bufs=1, space="PSUM") as psP:
            pd_ps = psP.tile([64, 85], f32, name="pd_ps")
            for s in (8, 1, 4, 2):
                po, qo, n = PDOFF[s], POFF[s], s * s
                if s == 1:
                    k = 0
                    for sl in range(2):
                        for j in range(2):
                            nc.tensor.matmul(pd_ps[:, po: po + 1],
                                             psp_wT[:, j, SI[s], :],
                                             slp[sl][:, j, qo: qo + 1],
                                             start=(k == 0), stop=(k == 3))
                            k += 1
                else:
                    half = n // 2
                    for sl in range(2):
                        for j in range(2):
                            nc.tensor.matmul(
                                pd_ps[:, po + sl * half: po + sl * half + half],
                                psp_wT[:, j, SI[s], :],
                                slp[sl][:, j, qo: qo + half],
                                start=(j == 0), stop=(j == 1))
            nc.scalar.copy(pd[:, :], pd_ps[:, :])

        if stage == "dbg":
            nc.sync.dma_start(prm["dbg_pd"][:], pd[:])

        # upsample via PE transpose of pd + folded operators Wup
        ones_f = wk.tile([128, 512], f32, name="ones_f")
        nc.vector.memset(ones_f[:], 1.0)
        pdT_a = wk.tile([65, 64], f32, name="pdT_a")
        pdT_b = wk.tile([16, 64], f32, name="pdT_b")
        pdT_c = wk.tile([4, 64], f32, name="pdT_c")
        pri = [wk.tile([128, 512], f16, tag=f"pri{i}", name=f"pri{i}")
               for i in range(2)]
        with tc.tile_pool(name="psQ", bufs=2, space="PSUM") as psQ:
            pta = psQ.tile([65, 64], f32, tag="pta", name="pta")
            nc.tensor.transpose(pta[:, :], pd[:, 0:65], ident[0:64, 0:64])
            nc.vector.tensor_copy(pdT_a[:, :], pta[:, :])
            ptb = psQ.tile([16, 64], f32, tag="ptb", name="ptb")
            nc.tensor.transpose(ptb[:, :], pd[:, 65:81], ident[0:64, 0:64])
            nc.vector.tensor_copy(pdT_b[:, :], ptb[:, :])
            ptc = psQ.tile([4, 64], f32, tag="ptc", name="ptc")
            nc.tensor.transpose(ptc[:, :], pd[:, 81:85], ident[0:64, 0:64])
            nc.vector.tensor_copy(pdT_c[:, :], ptc[:, :])
        with tc.tile_pool(name="psR", bufs=2, space="PSUM") as psR:
            pp0 = psR.tile([128, 512], f32, tag="pp", name="pp0")
            nc.tensor.matmul(pp0[0:64, :], pdT_a[64:65, :], ones_f[64:65, :],
                             start=True, stop=True, tile_position=(64, 0))
            nc.tensor.matmul(pp0[64:128, :], pdT_c[0:4, :], Wup[0:4, 0, :],
                             start=True, stop=True, tile_position=(0, 64))
            nc.scalar.copy(pri[0][:, :], pp0[:, :])
            pp1 = psR.tile([128, 512], f32, tag="pp", name="pp1")
            nc.tensor.matmul(pp1[0:64, :], pdT_b[0:16, :], Wup[0:16, 1, :],
                             start=True, stop=True)
            nc.tensor.matmul(pp1[64:128, :], pdT_a[0:64, :], Wup[0:64, 2, :],
                             start=True, stop=True, tile_position=(0, 64))
            nc.scalar.copy(pri[1][:, :], pp1[:, :])

        if stage == "dbg":
            nc.sync.dma_start(prm["dbg_pri0"][:], pri[0][:])
            nc.sync.dma_start(prm["dbg_pri1"][:], pri[1][:])

        out_sb = wk.tile([128, 2, 512], f32, name="out_sb")
        fbv = fused.rearrange("p j r c -> p j (r c)")
        rhs_chunks = [pri[0][:, :], pri[1][:, :], fbv[:, 0, :], fbv[:, 1, :]]
        with tc.tile_pool(name="psO", bufs=2, space="PSUM") as psO:
            for m in range(2):
                po = psO.tile([128, 512], f32, tag="po", name="po")
                for k in range(4):
                    nc.tensor.matmul(po[:, :],
                                     bott_wT[:, k, m * 128: m * 128 + 128],
                                     rhs_chunks[k],
                                     start=(k == 0), stop=(k == 3))
                nc.scalar.activation(out_sb[:, m, :], po[:, :], AF.Relu,
                                     bias=bott_b[:, m: m + 1])
        nc.sync.dma_start(out_prm[:],
                          out_sb.rearrange("p j (r c) -> p j r c", c=32))


# ---------------------------------------------------------------------------
# Runner
# ---------------------------------------------------------------------------

_CACHE = {}


def _get_nc(stage="full"):
    if stage not in _CACHE:
        _CACHE[stage] = build(stage)
    return _CACHE[stage]


def run_cores(inputs, stage="full"):
    nc = _get_nc(stage)
    in_maps = [prep_core_inputs(inputs, c) for c in range(N_CORES)]
    res = run_bass_kernel_spmd(nc, in_maps, list(range(N_CORES)))
    return res.results


def kernel(**inputs):
    results = run_cores(inputs, "full")
    out = np.zeros((B, 1, COUT, H, W), np.float32)
    for c in range(N_CORES):
        b, h = c // 2, c % 2
        o = results[c]["out"]                    # [128, 2, 16, 32]
        out[b, 0, :, 16 * h: 16 * h + 16, :] = (
            o.transpose(1, 0, 2, 3).reshape(COUT, 16, 32))
    return out

